# revision 2
# baseline (speedup 1.0000x reference)
"""Trainium2 Bass kernel v2 for nn_MultiHeadAttention_34144990003301.

Head-parallel attention (2 heads/core, BN stats local), bf16 matmuls
throughout, BN1 folded into the para_linear1 epilogue (no separate
apply pass), chunked bf16 AllGather of the raw attention output
overlapped with the attention token loop, hid-sharded (625/core)
para_linear1, W2 partials AllReduced, sigmoid on device.
"""

import numpy as np

BS, HEADS, FN, SL, KN, ST = 32, 16, 124, 256, 64, 4
HID = 5000
EPS = 1e-5
SLOPE = 0.01
N_CORES = 8
HL = HEADS // N_CORES          # 2 local heads per core
ROWS = HL * KN                 # 128 window rows (aligned per-head 64-row slabs)
TL = BS * HL                   # 64 local tokens
T = BS * HEADS                 # 512 global tokens
HSH = HID // N_CORES           # 625 hid cols per core
IC = SL // 128                 # 2 i-chunks
NKT = SL * KN // 128           # 128 k-tiles for the para_linear1 matmul
HCH = [128, 128, 128, 128, HSH - 4 * 128]
G = 2                          # gather chunks (32 tokens each)
CH = (TL // G) * 128           # 2048 cols per gather chunk

_prog = None


def _build():
    import concourse.bacc as bacc
    import concourse.tile as tile
    import concourse.mybir as mybir

    f32 = mybir.dt.float32
    f32r = mybir.dt.float32r
    bf16 = mybir.dt.bfloat16
    AF = mybir.ActivationFunctionType
    OP = mybir.AluOpType
    RG = [list(range(N_CORES))]

    nc = bacc.Bacc("TRN2", target_bir_lowering=False, debug=False,
                   num_devices=N_CORES)

    def din(name, shape, dt=None):
        return nc.dram_tensor(
            name, list(shape), dt or f32, kind="ExternalInput"
        ).ap()

    q_d = din("qh", (FN, BS * SL), bf16)
    k_d = din("kh", (FN, BS * SL), bf16)
    v_d = din("vh", (FN, BS * SL), bf16)
    wq_d = din("wqT", (FN, ROWS), bf16)
    wk_d = din("wkT", (FN, ROWS), bf16)
    wv_d = din("wvT", (FN, ROWS), bf16)
    bq_d = din("bq", (ROWS,))
    bk_d = din("bk", (ROWS,))
    bv_d = din("bv", (ROWS,))
    bnp_d = din("bnp", (HL, 8))      # [hl, (gq,beq,gk,bek,gv,bev,g1,be1)]
    bnp1_d = din("bnp1", (1, 2 * HL))  # (g1_h0, be1_h0, g1_h1, be1_h1)
    mask_d = din("mask68", (ROWS, HL))
    sel2_d = din("sel2", (HL, 128))
    eye_d = din("eye64", (2 * KN, KN), bf16)  # [I64; I64] stacked
    selr_d = din("selr", (HEADS, T), f32r)
    w1_d = din("w1T", (SL * KN, HSH), bf16)
    w1s2_d = din("w1s2", (2, HSH), f32r)   # row0 = W1 col-sums, row1 = b1
    b1_d = din("b1s", (HSH,))
    ones16_d = din("ones16r", (HEADS, 128), f32r)
    onecol_d = din("onecol", (128, 1), bf16)
    w2_d = din("w2T", (HSH, KN), bf16)
    b2_d = din("b2", (KN,))
    out_d = nc.dram_tensor("out", [KN, T], f32, kind="ExternalOutput").ap()

    with tile.TileContext(nc) as tc:
        with (
            tc.tile_pool(name="persist", bufs=1) as pp,
            tc.tile_pool(name="dram", bufs=1, space="DRAM") as dp,
        ):
            # ---------- constants ----------
            bq_sb = pp.tile([ROWS, 1], f32, tag="bq")
            nc.gpsimd.dma_start(bq_sb[:], bq_d.unsqueeze(1))
            bk_sb = pp.tile([ROWS, 1], f32, tag="bk")
            nc.gpsimd.dma_start(bk_sb[:], bk_d.unsqueeze(1))
            bv_sb = pp.tile([ROWS, 1], f32, tag="bv")
            nc.gpsimd.dma_start(bv_sb[:], bv_d.unsqueeze(1))
            bnp_sb = pp.tile([HL, 8], f32, tag="bnp")
            nc.gpsimd.dma_start(bnp_sb[:], bnp_d)
            bnp1_sb = pp.tile([1, 2 * HL], f32, tag="bnp1")
            nc.gpsimd.dma_start(bnp1_sb[:], bnp1_d)
            mask_sb = pp.tile([ROWS, HL], f32, tag="mask")
            nc.gpsimd.dma_start(mask_sb[:], mask_d)
            sel2_sb = pp.tile([HL, 128], f32, tag="sel2")
            nc.gpsimd.dma_start(sel2_sb[:], sel2_d)
            eye_sb = pp.tile([2 * KN, KN], bf16, tag="eye")
            nc.gpsimd.dma_start(eye_sb[:], eye_d)
            selr_sb = pp.tile([HEADS, T], f32r, tag="selr")
            nc.gpsimd.dma_start(selr_sb[:], selr_d)
            w1s2_sb = pp.tile([2, HSH], f32r, tag="w1s2")
            nc.gpsimd.dma_start(w1s2_sb[:], w1s2_d)
            b2_sb = pp.tile([KN, 1], f32, tag="b2")
            nc.gpsimd.dma_start(b2_sb[:], b2_d.unsqueeze(1))
            w2_sb = []
            b1_sb = []
            for j in range(5):
                c0 = j * 128
                t2_ = pp.tile([HCH[j], KN], bf16, tag=f"w2_{j}")
                nc.gpsimd.dma_start(t2_[:], w2_d[c0:c0 + HCH[j], :])
                w2_sb.append(t2_)
                tb_ = pp.tile([HCH[j], 1], f32, tag=f"b1_{j}")
                nc.gpsimd.dma_start(tb_[:], b1_d[c0:c0 + HCH[j]].unsqueeze(1))
                b1_sb.append(tb_)
            ones128c = pp.tile([128, 1], f32, tag="ones128c")
            nc.vector.memset(ones128c[:], 1.0 / 128.0)
            ones16 = pp.tile([HEADS, 128], f32r, tag="ones16")
            nc.gpsimd.dma_start(ones16[:], ones16_d)

            # persistent big tensors
            qp = pp.tile([ROWS, BS * SL], bf16, tag="qp")
            kp = pp.tile([ROWS, BS * SL], bf16, tag="kp")
            vp = pp.tile([ROWS, BS * SL], bf16, tag="vp")
            # O layout: [p, (ick, tl)] — token-minor for contiguous
            # gather chunks and contiguous Phase F reads
            O_sb = pp.tile([128, 128 * TL], bf16, tag="osb")
            O_sbv = O_sb[:].rearrange("p (ick tl) -> p ick tl", tl=TL)
            ab128 = pp.tile([128, 6], f32, tag="ab128")
            # vs slots: [128, 65] bf16, col 64 = 1.0 (softmax denom trick)
            vss = []
            for s in range(4):
                vsl = pp.tile([128, KN + 1], bf16, tag=f"vss{s}")
                nc.sync.dma_start(vsl[:, KN:KN + 1], onecol_d)
                vss.append(vsl)

            # DRAM comm buffers
            floc4 = dp.tile([G, 128, CH], bf16, tag="floc4")
            fgl4 = [
                dp.tile([N_CORES, 128, CH], bf16, tag=f"fgl4_{g}",
                        name=f"fgl4_{g}", addr_space="Shared")
                for g in range(G)
            ]
            csin = dp.tile([1, 4], f32, tag="csin")
            csgl = dp.tile([N_CORES, 1, 4], f32, tag="csgl")
            arin = dp.tile([KN, T], f32, tag="arin")
            arout = dp.tile([KN, T], f32, tag="arout")

            # ---------- Phase A: projections + BN stats (fused) ----------
            with (
                tc.tile_pool(name="xin", bufs=1) as xp,
                tc.tile_pool(name="psA", bufs=3, space="PSUM") as psA,
                tc.tile_pool(name="stA", bufs=1) as stA,
            ):
                stat2all = stA.tile([ROWS, 6], f32, tag="stat2all")
                for ti, (x_d, w_d, b_sb, dst) in enumerate((
                    (q_d, wq_d, bq_sb, qp),
                    (k_d, wk_d, bk_sb, kp),
                    (v_d, wv_d, bv_sb, vp),
                )):
                    x_sb = xp.tile([FN, BS * SL], bf16, tag=f"x{ti}")
                    half = BS * SL // 2
                    nc.sync.dma_start(x_sb[:, 0:half], x_d[:, 0:half])
                    nc.scalar.dma_start(x_sb[:, half:], x_d[:, half:])
                    w_sb = xp.tile([FN, ROWS], bf16, tag=f"w{ti}")
                    nc.sync.dma_start(w_sb[:], w_d)
                    bnst = stA.tile([ROWS, 16 * 6], f32, tag=f"bnst{ti}")
                    for n in range(16):
                        ps = psA.tile([ROWS, 512], f32, tag="proj")
                        nc.tensor.matmul(
                            ps[:], w_sb[:], x_sb[:, n * 512:(n + 1) * 512]
                        )
                        nc.vector.bn_stats(bnst[:, 6 * n:6 * (n + 1)], ps[:])
                        nc.scalar.activation(
                            dst[:, n * 512:(n + 1) * 512], ps[:],
                            AF.Prelu, bias=b_sb[:], scale=1.0, alpha=1.0,
                        )
                    mv = stA.tile([ROWS, 2], f32, tag=f"mv{ti}")
                    nc.vector.bn_aggr(
                        mv[:], bnst[:].rearrange("p (c s) -> p c s", s=6)
                    )
                    # mean of (Wx+b) = mean(Wx) + b
                    meanc = stA.tile([ROWS, 1], f32, tag=f"meanc{ti}")
                    nc.vector.tensor_tensor(
                        meanc[:], mv[:, 0:1], b_sb[:], op=OP.add
                    )
                    nc.vector.tensor_copy(
                        stat2all[:, 2 * ti:2 * ti + 1], meanc[:]
                    )
                    nc.vector.scalar_tensor_tensor(
                        stat2all[:, 2 * ti + 1:2 * ti + 2],
                        meanc[:], meanc[:], mv[:, 1:2],
                        op0=OP.mult, op1=OP.add,
                    )

                # combine rows -> per-head stats, then a/b coefficients
                with tc.tile_pool(name="psB", bufs=1, space="PSUM") as psB:
                    hs = psB.tile([HL, 6], f32, tag="hs")
                    nc.tensor.matmul(hs[:], mask_sb[:], stat2all[:])
                    hs_sb = stA.tile([HL, 6], f32, tag="hs_sb")
                    nc.vector.tensor_copy(hs_sb[:], hs[:])
                    hsv = hs_sb[:].rearrange("p (c s) -> p s c", s=2)
                    means, msqs = hsv[:, 0, :], hsv[:, 1, :]
                    bnpv = bnp_sb[:, 0:6].rearrange("p (c s) -> p s c", s=2)
                    gam, bet = bnpv[:, 0, :], bnpv[:, 1, :]
                    m2 = stA.tile([HL, 3], f32, tag="m2")
                    nc.vector.tensor_tensor(m2[:], means, means, op=OP.mult)
                    var3 = stA.tile([HL, 3], f32, tag="var3")
                    nc.vector.tensor_tensor(var3[:], msqs, m2[:], op=OP.subtract)
                    nc.vector.tensor_scalar_add(var3[:], var3[:], EPS)
                    rv3 = stA.tile([HL, 3], f32, tag="rv3")
                    nc.vector.reciprocal(rv3[:], var3[:])
                    rsq3 = stA.tile([HL, 3], f32, tag="rsq3")
                    nc.scalar.sqrt(rsq3[:], rv3[:])
                    a3 = stA.tile([HL, 3], f32, tag="a3")
                    nc.vector.tensor_tensor(a3[:], gam, rsq3[:], op=OP.mult)
                    tm3 = stA.tile([HL, 3], f32, tag="tm3")
                    nc.vector.tensor_tensor(tm3[:], means, a3[:], op=OP.mult)
                    b3 = stA.tile([HL, 3], f32, tag="b3")
                    nc.vector.tensor_tensor(b3[:], bet, tm3[:], op=OP.subtract)
                    ab6 = stA.tile([HL, 6], f32, tag="ab6")
                    ab6v = ab6[:].rearrange("p (c s) -> p s c", s=2)
                    nc.vector.tensor_copy(ab6v[:, 0, :], a3[:])
                    nc.vector.tensor_copy(ab6v[:, 1, :], b3[:])
                    abps = psB.tile([128, 6], f32, tag="abps")
                    nc.tensor.matmul(abps[:], sel2_sb[:], ab6[:])
                    nc.vector.tensor_copy(ab128[:], abps[:])

            # ---------- Phase C: attention, 2 heads packed, chunked gather
            with (
                tc.tile_pool(name="stage", bufs=3) as sg,
                tc.tile_pool(name="expp", bufs=6) as ep,
                tc.tile_pool(name="small", bufs=4) as smp,
                tc.tile_pool(name="ps_sc", bufs=4, space="PSUM") as pssc,
                tc.tile_pool(name="ps_vt", bufs=2, space="PSUM") as psvt,
                tc.tile_pool(name="ps_uo", bufs=2, space="PSUM") as psuo,
            ):
                for b in range(BS):
                    bsl = slice(b * SL, (b + 1) * SL)
                    qw2 = sg.tile([128, SL], bf16, tag="qw2")
                    nc.vector.tensor_scalar(
                        qw2[:], qp[:, bsl], ab128[:, 0:1], ab128[:, 1:2],
                        op0=OP.mult, op1=OP.add,
                    )
                    kw2 = sg.tile([128, SL], bf16, tag="kw2")
                    nc.vector.tensor_scalar(
                        kw2[:], kp[:, bsl], ab128[:, 2:3], ab128[:, 3:4],
                        op0=OP.mult, op1=OP.add,
                    )
                    vw2 = sg.tile([128, SL], bf16, tag="vw2")
                    nc.vector.tensor_scalar(
                        vw2[:], vp[:, bsl], ab128[:, 4:5], ab128[:, 5:6],
                        op0=OP.mult, op1=OP.add,
                    )
                    for hl in range(HL):
                        p0 = KN * hl
                        tl = HL * b + hl
                        eTs = []
                        vsl2 = []
                        for jc in range(2):
                            scT = pssc.tile([128, SL], f32, tag="scT")
                            nc.tensor.matmul(
                                scT[:],
                                kw2[p0:p0 + KN, jc * 128:(jc + 1) * 128],
                                qw2[p0:p0 + KN, :],
                            )
                            eT = ep.tile([128, SL], bf16, tag="expT")
                            nc.scalar.activation(
                                eT[:], scT[:], AF.Exp, bias=0.0, scale=0.125
                            )
                            eTs.append(eT)
                            vt = psvt.tile([128, KN], bf16, tag="vt")
                            nc.tensor.transpose(
                                vt[:], vw2[p0:p0 + KN, jc * 128:(jc + 1) * 128],
                                eye_sb[p0:p0 + KN, :],
                            )
                            vsl = vss[(tl * 2 + jc) % 4]
                            nc.vector.tensor_copy(vsl[:, 0:KN], vt[:])
                            vsl2.append(vsl)
                        for ic in range(IC):
                            uo = psuo.tile([128, KN + 1], f32, tag="uo")
                            for jc in range(2):
                                nc.tensor.matmul(
                                    uo[:],
                                    eTs[jc][:, ic * 128:(ic + 1) * 128],
                                    vsl2[jc][:],
                                    start=(jc == 0), stop=(jc == 1),
                                )
                            rec = smp.tile([128, 1], f32, tag="rec")
                            nc.vector.reciprocal(rec[:], uo[:, KN:KN + 1])
                            if ic == 0:
                                nc.vector.tensor_scalar(
                                    O_sbv[:, ic * KN:(ic + 1) * KN, tl],
                                    uo[:, 0:KN], rec[:], 0.0,
                                    op0=OP.mult, op1=OP.add,
                                )
                            else:
                                nc.scalar.activation(
                                    O_sbv[:, ic * KN:(ic + 1) * KN, tl],
                                    uo[:, 0:KN], AF.Prelu,
                                    bias=0.0, scale=rec[:], alpha=1.0,
                                )
                    if (b + 1) % (BS // G) == 0:
                        g = b // (BS // G)
                        nc.sync.dma_start(
                            floc4[g].rearrange(
                                "p (ick tlc) -> p ick tlc", ick=128),
                            O_sbv[:, :, g * (TL // G):(g + 1) * (TL // G)],
                        )
                        nc.gpsimd.collective_compute(
                            "AllGather", OP.bypass, replica_groups=RG,
                            ins=[floc4[g].opt()], outs=[fgl4[g][:].opt()],
                        )

            # ---- BN1 stats over O (per local head) + tiny gather ----
            if True:
                with tc.tile_pool(name="stC", bufs=1) as stC:
                    scrap = stC.tile([128, TL * 64], bf16, tag="scrapC")
                    scrapv = scrap[:].rearrange(
                        "p (ick bb) -> p ick bb", ick=128
                    )
                    S4 = stC.tile([128, 4], f32, tag="S4")
                    Ov = O_sb[:].rearrange(
                        "p (ick bb two) -> p two ick bb", two=HL, ick=128
                    )
                    for hl in range(HL):
                        nc.vector.tensor_scalar(
                            scrapv, Ov[:, hl], 1.0, 0.0,
                            op0=OP.mult, op1=OP.add,
                            accum_out=S4[:, 2 * hl:2 * hl + 1],
                        )
                        nc.vector.scalar_tensor_tensor(
                            scrapv, Ov[:, hl], 1.0, Ov[:, hl],
                            op0=OP.bypass, op1=OP.mult,
                            accum_out=S4[:, 2 * hl + 1:2 * hl + 2],
                        )
                    msc = stC.tile([128, 4], f32, tag="msc")
                    nc.vector.tensor_scalar_mul(
                        msc[:], S4[:], 1.0 / (TL * 64)
                    )
                    with tc.tile_pool(name="psC", bufs=1, space="PSUM") as psC:
                        mps = psC.tile([1, 4], f32, tag="mps")
                        nc.tensor.matmul(mps[:], ones128c[:], msc[:])
                        ms = stC.tile([1, 4], f32, tag="ms")
                        nc.vector.tensor_copy(ms[:], mps[:])
                    msv = ms[:].rearrange("p (h s) -> p s h", s=2)
                    mean2, msq2 = msv[:, 0, :], msv[:, 1, :]
                    bn1v = bnp1_sb[:].rearrange("p (h s) -> p s h", s=2)
                    g1g, be1g = bn1v[:, 0, :], bn1v[:, 1, :]
                    mm2 = stC.tile([1, 2], f32, tag="mm2")
                    nc.vector.tensor_tensor(mm2[:], mean2, mean2, op=OP.mult)
                    var2 = stC.tile([1, 2], f32, tag="var2")
                    nc.vector.tensor_tensor(var2[:], msq2, mm2[:], op=OP.subtract)
                    nc.vector.tensor_scalar_add(var2[:], var2[:], EPS)
                    rv2 = stC.tile([1, 2], f32, tag="rv2")
                    nc.vector.reciprocal(rv2[:], var2[:])
                    rsq2 = stC.tile([1, 2], f32, tag="rsq2")
                    nc.scalar.sqrt(rsq2[:], rv2[:])
                    a1t = stC.tile([1, 2], f32, tag="a1t")
                    nc.vector.tensor_tensor(a1t[:], g1g, rsq2[:], op=OP.mult)
                    tm1 = stC.tile([1, 2], f32, tag="tm1")
                    nc.vector.tensor_tensor(tm1[:], mean2, a1t[:], op=OP.mult)
                    b1t = stC.tile([1, 2], f32, tag="b1t")
                    nc.vector.tensor_tensor(b1t[:], be1g, tm1[:], op=OP.subtract)
                    cs_loc = stC.tile([1, 4], f32, tag="cs_loc")
                    csv = cs_loc[:].rearrange("p (h s) -> p s h", s=2)
                    nc.vector.tensor_copy(csv[:, 0, :], a1t[:])
                    nc.vector.tensor_copy(csv[:, 1, :], b1t[:])
                    nc.sync.dma_start(csin[:], cs_loc[:])
                    nc.gpsimd.collective_compute(
                        "AllGather", OP.bypass, replica_groups=RG,
                        ins=[csin[:].opt()], outs=[csgl[:].opt()],
                    )

            # ---------- Phase F: h1 = A1*(O @ W1p) + W1sum*B1 + b1 ----------
            with (
                tc.tile_pool(name="w1p", bufs=3) as w1p,
                tc.tile_pool(name="rhp", bufs=3) as rhp,
                tc.tile_pool(name="h1sbp", bufs=1) as hp,
                tc.tile_pool(name="psH", bufs=1, space="PSUM") as psH,
                tc.tile_pool(name="psAD", bufs=1, space="PSUM") as psAD,
                tc.tile_pool(name="tpool", bufs=2) as tp,
            ):
                # A1/B1 row construction from gathered stats
                a1diag = hp.tile([HEADS, 2], f32, tag="a1diag")
                nc.sync.dma_start(
                    a1diag[:],
                    csgl[:].rearrange("c o (h s) -> (c h) (o s)", s=2),
                )
                repsA = hp.tile([HEADS, 128], f32r, tag="repsA")
                nc.vector.tensor_scalar(
                    repsA[:], ones16[:], a1diag[:, 0:1], 0.0,
                    op0=OP.mult, op1=OP.add,
                )
                repsB = hp.tile([HEADS, 128], f32r, tag="repsB")
                nc.vector.tensor_scalar(
                    repsB[:], ones16[:], a1diag[:, 1:2], 0.0,
                    op0=OP.mult, op1=OP.add,
                )
                A1ps = psAD.tile([128, T], f32, tag="scr", name="A1ps")
                nc.tensor.matmul(A1ps[:], repsA[:], selr_sb[:])
                A1row = hp.tile([128, T], f32r, tag="A1row")
                nc.vector.tensor_copy(A1row[:], A1ps[:])
                B1ps = psAD.tile([128, T], f32, tag="scr", name="B1ps")
                nc.tensor.matmul(B1ps[:], repsB[:], selr_sb[:])
                B1row = hp.tile([1, T], f32r, tag="B1row")
                nc.vector.tensor_copy(B1row[:], B1ps[0:1, :])

                TLC = TL // G  # 32 tokens per chunk
                h1ps = [
                    psH.tile([HCH[j], T], f32, tag=f"h1_{j}", name=f"h1ps_{j}")
                    for j in range(5)
                ]
                h1psv = [
                    h1ps[j][:].rearrange(
                        "o (c g tlc) -> o c g tlc", c=N_CORES, g=G
                    )
                    for j in range(5)
                ]
                # 16 groups of 8 consecutive ick rows; each group's data is a
                # contiguous 256-elem run per (c, p) in the gathered chunk
                for ikg in range(16):
                    rhbs = []
                    for g in range(G):
                        rhb = rhp.tile([128, N_CORES * 8 * TLC], bf16,
                                       tag=f"rhb{g}", name=f"rhb{g}_{ikg}")
                        qeng = nc.sync if g == 0 else nc.gpsimd
                        qeng.dma_start(
                            rhb[:].rearrange(
                                "p (c r) -> p c r", c=N_CORES),
                            fgl4[g][:].rearrange(
                                "c p f -> p c f"
                            )[:, :, ikg * 8 * TLC:(ikg + 1) * 8 * TLC],
                        )
                        rhbs.append(
                            rhb[:].rearrange(
                                "p (c ickl tlc) -> p c ickl tlc",
                                c=N_CORES, ickl=8)
                        )
                    for ickl in range(8):
                        kt = ikg * 8 + ickl
                        w1t = w1p.tile([128, HSH], bf16, tag="w1t")
                        nc.scalar.dma_start(
                            w1t[:], w1_d[kt * 128:(kt + 1) * 128, :]
                        )
                        for j in range(5):
                            for g in range(G):
                                nc.tensor.matmul(
                                    h1psv[j][:, :, g, :],
                                    w1t[:, j * 128:j * 128 + HCH[j]],
                                    rhbs[g][:, :, ickl, :],
                                    start=(kt == 0 and g == 0),
                                    stop=(kt == NKT - 1 and g == G - 1),
                                )
                h1sb = []
                for j in range(5):
                    c0 = j * 128
                    addf = psAD.tile([128, T], f32, tag="scr", name=f"addf{j}")[0:HCH[j], :]
                    nc.tensor.matmul(
                        addf[:], w1s2_sb[0:1, c0:c0 + HCH[j]], B1row[:]
                    )
                    t1 = tp.tile([HCH[j], T], f32r, tag="t1")
                    nc.vector.scalar_tensor_tensor(
                        t1[:], h1ps[j][:], 1.0, A1row[0:HCH[j], :],
                        op0=OP.bypass, op1=OP.mult,
                    )
                    t2_ = tp.tile([HCH[j], T], f32r, tag="t2")
                    nc.vector.scalar_tensor_tensor(
                        t2_[:], addf[:], 1.0, t1[:],
                        op0=OP.bypass, op1=OP.add,
                    )
                    h1c = hp.tile([HCH[j], T], bf16, tag=f"h1s_{j}")
                    nc.scalar.activation(
                        h1c[:], t2_[:], AF.Lrelu,
                        bias=b1_sb[j][:], scale=1.0, alpha=SLOPE,
                    )
                    h1sb.append(h1c)

                # ---------- W2 partial + AllReduce + sigmoid ------
                ps2 = psAD.tile([128, T], f32, tag="scr", name="ps2")[0:KN, :]
                for j in range(5):
                    nc.tensor.matmul(
                        ps2[:], w2_sb[j][:], h1sb[j][:],
                        start=(j == 0), stop=(j == 4),
                    )
                o2sb = hp.tile([KN, T], f32, tag="o2sb")
                nc.vector.tensor_copy(o2sb[:], ps2[:])
                nc.sync.dma_start(arin[:], o2sb[:])
                nc.gpsimd.collective_compute(
                    "AllReduce", OP.add, replica_groups=RG,
                    ins=[arin[:].opt()], outs=[arout[:].opt()],
                )
                arsb = hp.tile([KN, T], f32, tag="arsb")
                nc.sync.dma_start(arsb[:], arout[:])
                fin = hp.tile([KN, T], f32, tag="fin")
                nc.scalar.activation(
                    fin[:], arsb[:], AF.Sigmoid, bias=b2_sb[:], scale=1.0
                )
                nc.sync.dma_start(out_d, fin[:])

    nc.compile()
    return nc


def _dup_wT(W, c, bf):
    W = np.asarray(W, np.float32)
    cols = [W[8 * c + ST * hl: 8 * c + ST * hl + KN, :].T for hl in range(HL)]
    return np.ascontiguousarray(np.concatenate(cols, axis=1)).astype(bf)


def _dup_b(b, c):
    b = np.asarray(b, np.float32)
    rows = [b[8 * c + ST * hl: 8 * c + ST * hl + KN] for hl in range(HL)]
    return np.ascontiguousarray(np.concatenate(rows))


def _prep_in_maps(inputs):
    import ml_dtypes
    bf = ml_dtypes.bfloat16
    f = np.float32
    q = np.asarray(inputs["q"], f)
    k = np.asarray(inputs["k"], f)
    v = np.asarray(inputs["v"], f)
    qh = np.ascontiguousarray(
        q[:, 0].transpose(1, 0, 2).reshape(FN, BS * SL)).astype(bf)
    kh = np.ascontiguousarray(
        k[:, 0].transpose(1, 0, 2).reshape(FN, BS * SL)).astype(bf)
    vh = np.ascontiguousarray(
        v[:, 0].transpose(1, 0, 2).reshape(FN, BS * SL)).astype(bf)
    W1 = np.asarray(inputs["W1"], f)
    # columns permuted so device flat index ((ic*64+kk)*128+p) matches
    W1p = np.ascontiguousarray(
        W1.reshape(HID, IC, 128, KN).transpose(1, 3, 2, 0).reshape(SL * KN, HID)
    )
    W1pb = W1p.astype(bf)
    W1sum = W1pb.astype(f).sum(axis=0)  # [HID] col sums of bf16 weights
    b1f = np.asarray(inputs["b1"], f)
    W2T = np.ascontiguousarray(np.asarray(inputs["W2"], f).T)  # [5000, 64]
    mask = np.zeros((ROWS, HL), f)
    for hl in range(HL):
        mask[KN * hl:KN * (hl + 1), hl] = 1.0 / KN
    eye = np.concatenate([np.eye(KN, dtype=f)] * 2, axis=0).astype(bf)
    sel2 = np.zeros((HL, 128), f)
    for hl in range(HL):
        sel2[hl, hl * KN:(hl + 1) * KN] = 1.0
    # selr[h, t] = 1 iff global head of token t is h; t = c*64 + b*2 + hl
    selr = np.zeros((HEADS, T), f)
    tt = np.arange(T)
    selr[(tt // TL) * HL + (tt % HL), tt] = 1.0
    b2 = np.asarray(inputs["b2"], f)
    in_maps = []
    for c in range(N_CORES):
        h0 = HL * c
        bnp = np.stack(
            [
                np.array(
                    [
                        inputs["gq"][h0 + hl], inputs["beq"][h0 + hl],
                        inputs["gk"][h0 + hl], inputs["bek"][h0 + hl],
                        inputs["gv"][h0 + hl], inputs["bev"][h0 + hl],
                        inputs["g1"][h0 + hl], inputs["be1"][h0 + hl],
                    ],
                    dtype=f,
                )
                for hl in range(HL)
            ]
        )
        bnp1 = np.array(
            [[inputs["g1"][h0], inputs["be1"][h0],
              inputs["g1"][h0 + 1], inputs["be1"][h0 + 1]]], dtype=f
        )
        sh = slice(c * HSH, (c + 1) * HSH)
        w1s2 = np.ascontiguousarray(
            np.stack([W1sum[sh], b1f[sh]]).astype(f))
        m = {
            "qh": qh, "kh": kh, "vh": vh,
            "wqT": _dup_wT(inputs["Wq"], c, bf),
            "wkT": _dup_wT(inputs["Wk"], c, bf),
            "wvT": _dup_wT(inputs["Wv"], c, bf),
            "bq": _dup_b(inputs["bq"], c),
            "bk": _dup_b(inputs["bk"], c),
            "bv": _dup_b(inputs["bv"], c),
            "bnp": bnp, "bnp1": bnp1,
            "mask68": mask, "sel2": sel2, "eye64": eye, "selr": selr,
            "w1T": np.ascontiguousarray(W1pb[:, sh]),
            "w1s2": w1s2,
            "b1s": np.ascontiguousarray(b1f[sh]),
            "ones16r": np.ones((HEADS, 128), f),
            "onecol": np.ones((128, 1), f).astype(bf),
            "w2T": np.ascontiguousarray(W2T[sh, :]).astype(bf),
            "b2": b2,
        }
        in_maps.append(m)
    return in_maps


def kernel(**inputs):
    global _prog
    if _prog is None:
        _prog = _build()
    from concourse.bass_utils import run_bass_kernel_spmd

    in_maps = _prep_in_maps(inputs)
    res = run_bass_kernel_spmd(_prog, in_maps, list(range(N_CORES)))
    o = res.results[0]["out"]  # [KN, T], cols ordered (c, b, hl)
    out = (
        np.asarray(o, np.float32)
        .reshape(KN, N_CORES, BS, HL)
        .transpose(2, 1, 3, 0)
        .reshape(BS, HEADS, KN)[:, None]
    )
    return np.ascontiguousarray(out.astype(np.float32))
